# revision 15
# baseline (speedup 1.0000x reference)
"""MoE FFN (8 experts, top-2) on 8 Trainium2 NeuronCores.

Strategy: expert parallelism with host-side token routing.
  - Host computes the (tiny) gate: logits = x @ gate_w.T, top-2, softmax.
  - Tokens are gathered per expert and padded to a common capacity C.
  - Core e runs a dense FFN (gelu(x@W1[e].T+b1[e])@W2[e].T+b2[e]) over the
    C tokens routed to expert e, all in one SPMD Bass program.
  - Host scatters y back with the combine weights and sums the two
    expert contributions per token.

Device kernel layout (per core):
  FFN1: psum[inter128, tok] += W1T[k*128:, m*128:].T @ xT[k*128:, tok]
        h = gelu(psum + b1)           (ACT, writes bf16)
  FFN2: psum[hid128, tok]  += W2T[k*128:, m*128:].T @ h[k*128:, tok]
        y = psum + b2                 (DVE, writes f32)

DMA plumbing (v2): everything rides the two HWDGE rings (sync + scalar)
as a handful of large multi-engine DMAs.  Each dma_start is split across
all 16 SDMA engines (~340 GB/s), and each ring is FIFO in issue order,
which gives strict delivery priority: W1 column-phases then W2 k-phases
on sync; x tiles / biases / y outputs on scalar.  SWDGE (gpsimd) is not
used at all -- its Q7 descriptor rings live in SBUF and measurably slow
concurrent matmuls.  A burst of dummy matmuls at t=0 warms the PE HAM
clock (1.2 -> 2.4 GHz takes ~3.4 us of busy-ness) while the first loads
are in flight.
"""

import sys
import types

import numpy as np
import ml_dtypes

import concourse.bass as bass
import concourse.tile as tile
from concourse import mybir
from concourse.bass_utils import run_bass_kernel_spmd
from bass_rust import ScopedClock, VectorClock


def _ensure_axon_hooks():
    """run_bass_kernel_spmd(trace=True) under axon imports antenv.axon_hooks,
    which this image's antenv lacks.  Register an equivalent module backed by
    trn_agent_boot's ctypes NTFF hook so tracing works (and trace=False paths
    are unaffected)."""
    try:
        import antenv.axon_hooks  # noqa: F401
        return
    except ImportError:
        pass
    hook = None
    try:
        from trn_agent_boot.trn_boot import _ntff_profile_via_ctypes
        hook = _ntff_profile_via_ctypes("/opt/axon/libaxon_pjrt.so")
    except Exception:
        hook = None
    mod = types.ModuleType("antenv.axon_hooks")
    _state = {"hook": hook}
    mod.get_axon_ntff_profile_hook = lambda: _state["hook"]
    mod.set_axon_ntff_profile_hook = lambda h: _state.__setitem__("hook", h)
    sys.modules["antenv.axon_hooks"] = mod
    try:
        import antenv
        antenv.axon_hooks = mod
    except ImportError:
        pass


_ensure_axon_hooks()

H = 1024          # hidden
I = 4096          # intermediate
E = 8             # experts
NCORES = 8
KH = H // 128     # 8  k-tiles over hidden
KI = I // 128     # 32 k-tiles over inter
BF16 = mybir.dt.bfloat16
F32 = mybir.dt.float32

# W1 column-phases (over the 4096 inter cols).  Early phases are small so
# the first FFN1 psum-groups unblock quickly; each phase is ONE dma_start.
W1_PHASES = [(0, 128), (128, 384), (384, 640), (640, 1152), (1152, 2048),
             (2048, 3072), (3072, 4096)]
# W2 k-phases (over the 32 k-tiles of inter), consumed k-ascending.
W2_PHASES = [(0, 8), (8, 16), (16, 24), (24, 32)]

N_WARMUP_MM = 14  # 496-wide: ~3us of PE busy bridging preamble+first loads


class _TC(tile.TileContext):
    """TileContext whose tail drain splits its sem waits across SP nops.

    The walrus pinned in this container rejects a Drain instruction carrying
    more than a couple of sync waits ("Too many sync wait commands",
    CoreV3GenImpl.cpp:104).  Emit one wait-carrier nop per logical processor
    instead, then a waitless drain.
    """

    def _drain_and_barrier(self, tick_clock, wait_clock):
        nc = self.nc
        gc = tick_clock.global_clock
        ticks = eval(repr(gc).replace("VectorClock(", "").rstrip(")"))
        for i, t in enumerate(ticks):
            if t > 0:
                partial = [0] * len(ticks)
                partial[i] = t
                carrier = nc.sync.nop(nofuse=True, hint=f"drain_wait_{i}")
                wait_clock.add_sem_waits(
                    carrier.ins, ScopedClock({None: VectorClock(partial)})
                )
        nc.sync.drain()
        nc.all_engine_barrier()
        assert self.sems is not None
        popped = nc._tile_sem_poison_stack.pop()
        assert popped is self._sem_poison
        nc.clear_and_free_semaphores(list(self.sems.allocated().values()))
        nc.all_engine_barrier()


def _split_waits(nc, maxw=1):
    """The pinned walrus rejects instructions carrying more than one
    embedded sync wait ("Too many sync wait commands").  Hoist excess waits
    onto freshly inserted same-engine nops placed directly before the
    instruction -- the engine sequencer executes them in order, so the
    semantics are identical."""
    for fn in nc.m.functions:
        for bb in fn.blocks:
            new = []
            changed = False
            for inst in bb.instructions:
                si = inst.sync_info
                waits = list(si.on_wait) if si is not None else []
                if len(waits) > maxw:
                    changed = True
                    n_extra = len(waits) - maxw
                    for i in range(0, n_extra, maxw):
                        nop = mybir.InstNoOp(
                            name=nc.get_next_instruction_name(),
                            engine=inst.engine,
                            sync_info=mybir.SyncInfo(
                                on_wait=waits[i:i + maxw], on_update=[]
                            ),
                            bass_nofuse=True,
                        )
                        nc.register_instruction(nop, overwrite=True)
                        new.append(nop)
                    si.on_wait = waits[n_extra:]
                new.append(inst)
            if changed:
                bb.instructions = new


def _token_tiles(C):
    # Remainder tile last: the first (full) tile's FFN1 masks the W2 load.
    # 496-wide (not 512): a 512-col psum tile fills its bank exactly, which
    # measurably adds ~5-10 ns to every matmul in that group.
    tiles = [496] * (C // 496)
    if C % 496:
        tiles.append(C % 496)
    return tiles


def _w1_col_off(m):
    """SBUF col offset of W1 stationary block m (128 cols, one k) inside the
    phase-major w1all layout: phases concatenated, each phase laid out
    (k, cols-within-phase)."""
    off = 0
    for lo, hi in W1_PHASES:
        if m * 128 < hi:
            return off, hi - lo, m * 128 - lo
        off += KH * (hi - lo)
    raise AssertionError


def _build(C):
    """Dense per-expert FFN over C tokens; one SPMD program for all cores."""
    nc = bass.Bass()
    xt = nc.declare_dram_parameter("xt", [H, C], BF16, isOutput=False)
    w1t = nc.declare_dram_parameter("w1t", [H, I], BF16, isOutput=False)
    w2t = nc.declare_dram_parameter("w2t", [I, H], BF16, isOutput=False)
    b1 = nc.declare_dram_parameter("b1", [128, KI], F32, isOutput=False)
    b2 = nc.declare_dram_parameter("b2", [128, KH], F32, isOutput=False)
    yt = nc.declare_dram_parameter("yt", [H, C], F32, isOutput=True)

    # 3D views for single-DMA phase loads: (p, k, c) with k the 128-row block.
    w1v = w1t.rearrange("(k p) c -> p k c", k=KH)     # [128, 8, 4096]
    w2v = w2t.rearrange("(k p) c -> p k c", k=KI)     # [128, 32, 1024]
    xv = xt.rearrange("(k p) t -> p k t", k=KH)       # [128, 8, C]

    with _TC(nc) as tc:
        with (
            tc.tile_pool(name="weights", bufs=1) as wpool,
            tc.tile_pool(name="bias", bufs=1) as bpool,
            tc.tile_pool(name="x", bufs=3) as xpool,
            tc.tile_pool(name="h", bufs=1) as hpool,
            tc.tile_pool(name="o", bufs=4) as opool,
            tc.tile_pool(name="ps1", bufs=4, space="PSUM") as ps1pool,
            tc.tile_pool(name="ps2", bufs=4, space="PSUM") as ps2pool,
        ):
            # --- PE warmup: dummy matmuls on a zeroed tile so the HAM clock
            # ramps (1.2 -> 2.4 GHz) while the first real loads land.
            warm = wpool.tile([128, 624], BF16, tag="warm")
            nc.vector.memset(warm[:], 0.0)
            psw = ps1pool.tile([128, 496], F32, tag="ps1")
            for _ in range(N_WARMUP_MM):
                nc.tensor.matmul(psw[:], warm[:, 496:624], warm[:, 0:496],
                                 start=True, stop=True)

            # --- scalar(ACT) HWDGE ring: x tiles + biases (FIFO order).
            tiles = _token_tiles(C)
            xs = []

            def load_x(ti, nsplit=1):
                tw = tiles[ti]
                off = sum(tiles[:ti])
                t = xpool.tile([128, KH * tw], BF16, tag="xt")
                tv = t[:].rearrange("p (k t) -> p k t", k=KH)
                step = KH // nsplit
                for s in range(nsplit):
                    nc.scalar.dma_start(
                        tv[:, s * step:(s + 1) * step, :],
                        xv[:, s * step:(s + 1) * step, off:off + tw],
                    )
                xs.append(t)

            b1s = bpool.tile([128, KI], F32, tag="b1")
            b2s = bpool.tile([128, KH], F32, tag="b2")
            # First x tile split in four so FFN1 starts on quarter delivery;
            # b1 interleaved (needed at the first GELU, ~2us after MM 0).
            # x1/x2/b2 launches are deferred into tile-0's FFN1 so their
            # transfers don't steal HBM bandwidth from the W1 phase stream.
            tw0 = tiles[0]
            x0 = xpool.tile([128, KH * tw0], BF16, tag="xt")
            x0v = x0[:].rearrange("p (k t) -> p k t", k=KH)
            nc.scalar.dma_start(x0v[:, 0:2, :], xv[:, 0:2, 0:tw0])
            nc.scalar.dma_start(x0v[:, 2:4, :], xv[:, 2:4, 0:tw0])
            nc.scalar.dma_start(b1s[:], b1[:])
            nc.scalar.dma_start(x0v[:, 4:6, :], xv[:, 4:6, 0:tw0])
            nc.scalar.dma_start(x0v[:, 6:8, :], xv[:, 6:8, 0:tw0])
            xs.append(x0)

            # --- sync(SP) HWDGE ring: W1 column-phases then W2 k-phases.
            # Phase-major SBUF layout keeps every phase write contiguous
            # (exact dependency ranges) and every stationary block contiguous
            # (FWL-friendly).  The first two phases are split k-wise so the
            # first FFN1 psum-groups unblock on partial delivery.
            w1all = wpool.tile([128, KH * I], BF16, tag="w1")
            for pi, (lo, hi) in enumerate(W1_PHASES):
                off = sum(KH * (h_ - l_) for l_, h_ in W1_PHASES
                          if (l_, h_) < (lo, hi))
                pw = hi - lo
                nk = 2 if pi < 2 else 1
                kstep = KH // nk
                for s in range(nk):
                    dst = w1all[:, off + s * kstep * pw:
                                off + (s + 1) * kstep * pw].rearrange(
                        "p (k c) -> p k c", k=kstep)
                    nc.sync.dma_start(
                        dst, w1v[:, s * kstep:(s + 1) * kstep, lo:hi])
            w2all = wpool.tile([128, KI * H], BF16, tag="w2")
            for klo, khi in W2_PHASES:
                dst = w2all[:, klo * H:khi * H].rearrange(
                    "p (k c) -> p k c", k=khi - klo)
                nc.sync.dma_start(dst, w2v[:, klo:khi, :])

            def w1_stat(k, m):
                off, pw, rel = _w1_col_off(m)
                base = off + k * pw + rel
                return w1all[:, base:base + 128]

            off = 0
            for ti, tw in enumerate(tiles):
                xst = xs[ti]
                ht = hpool.tile([128, KI * tw], BF16, tag="h")
                for m in range(KI):
                    ps = ps1pool.tile([128, tw], F32, tag="ps1")
                    for k in range(KH):
                        nc.tensor.matmul(
                            ps[:],
                            w1_stat(k, m),
                            xst[:, k * tw:(k + 1) * tw],
                            start=(k == 0),
                            stop=(k == KH - 1),
                        )
                    nc.scalar.activation(
                        ht[:, m * tw:(m + 1) * tw],
                        ps[:],
                        mybir.ActivationFunctionType.Gelu,
                        bias=b1s[:, m:m + 1],
                    )
                    if ti == 0 and m == 8:
                        if len(tiles) > 1:
                            load_x(1)
                        nc.scalar.dma_start(b2s[:], b2[:])
                    if ti == 0 and m == 16 and len(tiles) > 2:
                        load_x(2)
                # Prefetch x for tile ti+3 AFTER this tile's FFN1: its
                # buffer WAR (xs[ti]'s last FFN1 read) is resolved by now,
                # so it doesn't block the scalar queue (GELUs/yt behind it).
                if ti + 3 <= len(tiles) - 1:
                    load_x(ti + 3)
                for m in range(KH):
                    ps = ps2pool.tile([128, tw], F32, tag="ps2")
                    for k in range(KI):
                        nc.tensor.matmul(
                            ps[:],
                            w2all[:, k * H + m * 128:k * H + (m + 1) * 128],
                            ht[:, k * tw:(k + 1) * tw],
                            start=(k == 0),
                            stop=(k == KI - 1),
                        )
                    ot = opool.tile([128, tw], F32, tag="o")
                    nc.vector.tensor_scalar_add(ot[:], ps[:], b2s[:, m:m + 1])
                    if ti == len(tiles) - 1 and m == KH - 1:
                        hw = tw // 2
                        nc.scalar.dma_start(
                            yt[m * 128:(m + 1) * 128, off:off + hw],
                            ot[:, :hw])
                        nc.sync.dma_start(
                            yt[m * 128:(m + 1) * 128, off + hw:off + tw],
                            ot[:, hw:])
                    else:
                        nc.scalar.dma_start(
                            yt[m * 128:(m + 1) * 128, off:off + tw], ot[:]
                        )
                off += tw
    _split_waits(nc)
    return nc


def _split_tiles(L):
    """Split a segment of L tokens into matmul tile widths.

    First tile 512 (masks the initial weight-phase streaming: FFN1 consumes
    W1 m-blocks slowest on a wide tile), last tile as big as possible (its
    FFN2 is the window that hides the next segment's W1 reload), middles
    >=128 (tiles narrower than ~128 risk pacing on LDWEIGHTS)."""
    if L <= 496:
        return [L]
    parts = [496]
    rem = L - 496
    while rem > 496:
        w = min(496, rem - 128)
        parts.append(w)
        rem -= w
    parts.append(rem)
    # first stays 512; order the rest ascending so the last is biggest
    return [parts[0]] + sorted(parts[1:])


def _plan_two_seg(cnts):
    """Two-segment expert-parallel plan: every core processes LA tokens of
    one expert then LB of another (weights reloaded mid-program), with
    (LA, LB) shared across cores (SPMD).  The busiest expert spans two
    A-slots, the lightest two B-slots, everyone else gets one A + one B:
      2*LA >= c_max,  LA+LB >= c_2nd,  2*LB >= c_min.
    Returns (LA, LB, slots) where slots[c] = ((eA, startA, lenA),
    (eB, startB, lenB)), or None when not profitable."""
    order = sorted(range(E), key=lambda e: -cnts[e])
    c = [cnts[e] for e in order]
    LA = -(-c[0] // 2)
    LB = max(-(-c[-1] // 2), c[1] - LA)
    LA = -(-LA // 8) * 8
    LB = max(128, -(-LB // 8) * 8)
    C1 = max(128, -(-c[0] // 128) * 128)          # single-segment capacity
    if LA + LB >= C1 or LA < 128:
        return None
    emax, emin = order[0], order[-1]
    mids = order[1:-1]                            # 6 middle experts
    a_slots = [(emax, 0), (emax, LA)] + [(e, 0) for e in mids]
    b_slots = [(e, LA) for e in mids] + [(emin, 0), (emin, LB)]
    slots = []
    for ci in range(NCORES):
        eA, sA = a_slots[ci]
        eB, sB = b_slots[ci]
        lA = max(0, min(LA, cnts[eA] - sA))
        lB = max(0, min(LB, cnts[eB] - sB))
        slots.append(((eA, sA, lA), (eB, sB, lB)))
    return LA, LB, slots


def _build_two_seg(LA, LB):
    """Per-core: segment A (LA tokens, expert A weights) then segment B
    (LB tokens, expert B weights).  B's weights stream into the same SBUF
    tiles during A's tail (WAR deps resolve per phase as A's last FFN1/FFN2
    march through the col/k ranges)."""
    tilesA = _split_tiles(LA)
    tilesB = _split_tiles(LB)
    tiles = tilesA + tilesB
    nseg_a = len(tilesA)
    C = LA + LB

    nc = bass.Bass()
    xt = nc.declare_dram_parameter("xt", [H, C], BF16, isOutput=False)
    w1ta = nc.declare_dram_parameter("w1ta", [H, I], BF16, isOutput=False)
    w2ta = nc.declare_dram_parameter("w2ta", [I, H], BF16, isOutput=False)
    b1a = nc.declare_dram_parameter("b1a", [128, KI], F32, isOutput=False)
    b2a = nc.declare_dram_parameter("b2a", [128, KH], F32, isOutput=False)
    w1tb = nc.declare_dram_parameter("w1tb", [H, I], BF16, isOutput=False)
    w2tb = nc.declare_dram_parameter("w2tb", [I, H], BF16, isOutput=False)
    b1b = nc.declare_dram_parameter("b1b", [128, KI], F32, isOutput=False)
    b2b = nc.declare_dram_parameter("b2b", [128, KH], F32, isOutput=False)
    yt = nc.declare_dram_parameter("yt", [H, C], F32, isOutput=True)

    w1va = w1ta.rearrange("(k p) c -> p k c", k=KH)
    w2va = w2ta.rearrange("(k p) c -> p k c", k=KI)
    w1vb = w1tb.rearrange("(k p) c -> p k c", k=KH)
    w2vb = w2tb.rearrange("(k p) c -> p k c", k=KI)
    xv = xt.rearrange("(k p) t -> p k t", k=KH)

    with _TC(nc) as tc:
        with (
            tc.tile_pool(name="weights", bufs=1) as wpool,
            tc.tile_pool(name="bias", bufs=1) as bpool,
            tc.tile_pool(name="x", bufs=3) as xpool,
            tc.tile_pool(name="h", bufs=1) as hpool,
            tc.tile_pool(name="o", bufs=4) as opool,
            tc.tile_pool(name="ps1", bufs=4, space="PSUM") as ps1pool,
            tc.tile_pool(name="ps2", bufs=4, space="PSUM") as ps2pool,
        ):
            warm = wpool.tile([128, 624], BF16, tag="warm")
            nc.vector.memset(warm[:], 0.0)
            psw = ps1pool.tile([128, 496], F32, tag="ps1")
            for _ in range(N_WARMUP_MM):
                nc.tensor.matmul(psw[:], warm[:, 496:624], warm[:, 0:496],
                                 start=True, stop=True)

            xs = []

            def load_x(ti, nsplit=1):
                tw = tiles[ti]
                off = sum(tiles[:ti])
                t = xpool.tile([128, KH * tw], BF16, tag="xt")
                tv = t[:].rearrange("p (k t) -> p k t", k=KH)
                step = KH // nsplit
                for s in range(nsplit):
                    nc.scalar.dma_start(
                        tv[:, s * step:(s + 1) * step, :],
                        xv[:, s * step:(s + 1) * step, off:off + tw],
                    )
                xs.append(t)

            b1sa = bpool.tile([128, KI], F32, tag="b1a")
            b2sa = bpool.tile([128, KH], F32, tag="b2a")
            b1sb = bpool.tile([128, KI], F32, tag="b1b")
            b2sb = bpool.tile([128, KH], F32, tag="b2b")
            tw0 = tiles[0]
            x0 = xpool.tile([128, KH * tw0], BF16, tag="xt")
            x0v = x0[:].rearrange("p (k t) -> p k t", k=KH)
            nc.scalar.dma_start(x0v[:, 0:2, :], xv[:, 0:2, 0:tw0])
            nc.scalar.dma_start(x0v[:, 2:4, :], xv[:, 2:4, 0:tw0])
            nc.scalar.dma_start(b1sa[:], b1a[:])
            nc.scalar.dma_start(x0v[:, 4:6, :], xv[:, 4:6, 0:tw0])
            nc.scalar.dma_start(x0v[:, 6:8, :], xv[:, 6:8, 0:tw0])
            xs.append(x0)

            w1all = wpool.tile([128, KH * I], BF16, tag="w1")
            w2all = wpool.tile([128, KI * H], BF16, tag="w2")

            def load_w(w1v, w2v, split_first=False):
                for pi, (lo, hi) in enumerate(W1_PHASES):
                    off = sum(KH * (h_ - l_) for l_, h_ in W1_PHASES
                              if (l_, h_) < (lo, hi))
                    pw = hi - lo
                    nk = 2 if (split_first and pi < 2) else 1
                    kstep = KH // nk
                    for s in range(nk):
                        dst = w1all[:, off + s * kstep * pw:
                                    off + (s + 1) * kstep * pw].rearrange(
                            "p (k c) -> p k c", k=kstep)
                        nc.sync.dma_start(
                            dst, w1v[:, s * kstep:(s + 1) * kstep, lo:hi])
                for klo, khi in W2_PHASES:
                    dst = w2all[:, klo * H:khi * H].rearrange(
                        "p (k c) -> p k c", k=khi - klo)
                    nc.sync.dma_start(dst, w2v[:, klo:khi, :])

            load_w(w1va, w2va, split_first=True)

            def w1_stat(k, m):
                off, pw, rel = _w1_col_off(m)
                base = off + k * pw + rel
                return w1all[:, base:base + 128]

            off = 0
            for ti, tw in enumerate(tiles):
                if ti == nseg_a:
                    # Segment B weights: WAR on segment A's last FFN1/FFN2
                    # reads resolves phase by phase; transfers hide under
                    # A's tail compute.
                    load_w(w1vb, w2vb)
                b1s, b2s = (b1sa, b2sa) if ti < nseg_a else (b1sb, b2sb)
                xst = xs[ti]
                ht = hpool.tile([128, KI * tw], BF16, tag="h")
                for m in range(KI):
                    ps = ps1pool.tile([128, tw], F32, tag="ps1")
                    for k in range(KH):
                        nc.tensor.matmul(
                            ps[:],
                            w1_stat(k, m),
                            xst[:, k * tw:(k + 1) * tw],
                            start=(k == 0),
                            stop=(k == KH - 1),
                        )
                    nc.scalar.activation(
                        ht[:, m * tw:(m + 1) * tw],
                        ps[:],
                        mybir.ActivationFunctionType.Gelu,
                        bias=b1s[:, m:m + 1],
                    )
                    if ti == 0 and m == 8:
                        if len(tiles) > 1:
                            load_x(1)
                        nc.scalar.dma_start(b2sa[:], b2a[:])
                    if ti == 0 and m == 16:
                        if len(tiles) > 2:
                            load_x(2)
                        nc.scalar.dma_start(b1sb[:], b1b[:])
                        nc.scalar.dma_start(b2sb[:], b2b[:])
                if ti + 3 <= len(tiles) - 1:
                    load_x(ti + 3)
                for m in range(KH):
                    ps = ps2pool.tile([128, tw], F32, tag="ps2")
                    for k in range(KI):
                        nc.tensor.matmul(
                            ps[:],
                            w2all[:, k * H + m * 128:k * H + (m + 1) * 128],
                            ht[:, k * tw:(k + 1) * tw],
                            start=(k == 0),
                            stop=(k == KI - 1),
                        )
                    ot = opool.tile([128, tw], F32, tag="o")
                    nc.vector.tensor_scalar_add(ot[:], ps[:], b2s[:, m:m + 1])
                    if ti == len(tiles) - 1 and m == KH - 1:
                        hw = tw // 2
                        nc.scalar.dma_start(
                            yt[m * 128:(m + 1) * 128, off:off + hw],
                            ot[:, :hw])
                        nc.sync.dma_start(
                            yt[m * 128:(m + 1) * 128, off + hw:off + tw],
                            ot[:, hw:])
                    else:
                        nc.scalar.dma_start(
                            yt[m * 128:(m + 1) * 128, off:off + tw], ot[:]
                        )
                off += tw
    _split_waits(nc)
    return nc


def _route(x, gate_w):
    """Host gate: top-2 of 8 logits + softmax over the selected pair."""
    logits = x @ gate_w.T                         # [T, E] f32
    T = logits.shape[0]
    rows = np.arange(T)
    i1 = np.argmax(logits, axis=1)
    v1 = logits[rows, i1]
    masked = logits.copy()
    masked[rows, i1] = -np.inf
    i2 = np.argmax(masked, axis=1)
    v2 = masked[rows, i2]
    # softmax over (v1, v2) with v1 >= v2
    e2 = np.exp(v2 - v1)
    w1 = 1.0 / (1.0 + e2)
    w2 = 1.0 - w1
    return i1, i2, w1.astype(np.float32), w2.astype(np.float32)


def _run(inputs, trace=False):
    hidden_states = np.asarray(inputs["hidden_states"], dtype=np.float32)
    gate_w = np.asarray(inputs["gate_w"], dtype=np.float32)
    W1 = np.asarray(inputs["W1"], dtype=np.float32)
    b1 = np.asarray(inputs["b1"], dtype=np.float32)
    W2 = np.asarray(inputs["W2"], dtype=np.float32)
    b2 = np.asarray(inputs["b2"], dtype=np.float32)

    B, S, _ = hidden_states.shape
    T = B * S
    x = np.ascontiguousarray(hidden_states.reshape(T, H))

    i1, i2, w1, w2 = _route(x, gate_w)
    toks = [np.flatnonzero((i1 == e) | (i2 == e)) for e in range(E)]
    cnts = [len(t) for t in toks]

    xb = x.astype(ml_dtypes.bfloat16)
    w1b = [np.ascontiguousarray(W1[e].astype(ml_dtypes.bfloat16).T)
           for e in range(E)]
    w2b = [np.ascontiguousarray(W2[e].astype(ml_dtypes.bfloat16).T)
           for e in range(E)]
    b1r = [np.ascontiguousarray(b1[e].reshape(I // 128, 128).T)
           for e in range(E)]
    b2r = [np.ascontiguousarray(b2[e].reshape(H // 128, 128).T)
           for e in range(E)]

    out = np.zeros((T, H), dtype=np.float32)

    def combine_w(e, te):
        return np.where(i1[te] == e, w1[te], w2[te])

    plan = _plan_two_seg(cnts)
    if plan is not None:
        LA, LB, slots = plan
        nc = _build_two_seg(LA, LB)
        in_maps = []
        for (eA, sA, lA), (eB, sB, lB) in slots:
            xe = np.zeros((LA + LB, H), dtype=ml_dtypes.bfloat16)
            xe[:lA] = xb[toks[eA][sA:sA + lA]]
            xe[LA:LA + lB] = xb[toks[eB][sB:sB + lB]]
            in_maps.append(
                {
                    "xt": np.ascontiguousarray(xe.T),
                    "w1ta": w1b[eA], "w2ta": w2b[eA],
                    "b1a": b1r[eA], "b2a": b2r[eA],
                    "w1tb": w1b[eB], "w2tb": w2b[eB],
                    "b1b": b1r[eB], "b2b": b2r[eB],
                }
            )
        res = run_bass_kernel_spmd(
            nc, in_maps, core_ids=list(range(NCORES)), trace=trace
        )
        for c, ((eA, sA, lA), (eB, sB, lB)) in enumerate(slots):
            ytc = res.results[c]["yt"]
            if lA:
                te = toks[eA][sA:sA + lA]
                out[te] += combine_w(eA, te)[:, None] * ytc[:, :lA].T
            if lB:
                te = toks[eB][sB:sB + lB]
                out[te] += combine_w(eB, te)[:, None] * ytc[:, LA:LA + lB].T
        return out.reshape(B, S, H), res

    C = max(128, -(-max(cnts) // 128) * 128)
    nc = _build(C)
    in_maps = []
    for e in range(E):
        xe = np.zeros((C, H), dtype=ml_dtypes.bfloat16)
        xe[: cnts[e]] = xb[toks[e]]
        in_maps.append(
            {
                "xt": np.ascontiguousarray(xe.T),
                "w1t": w1b[e], "w2t": w2b[e],
                "b1": b1r[e], "b2": b2r[e],
            }
        )
    res = run_bass_kernel_spmd(
        nc, in_maps, core_ids=list(range(NCORES)), trace=trace
    )
    for e in range(E):
        te = toks[e]
        ye = res.results[e]["yt"][:, : cnts[e]].T          # [cnt, H]
        out[te] += combine_w(e, te)[:, None] * ye
    return out.reshape(B, S, H), res


def kernel(**inputs):
    out, _ = _run(inputs, trace=False)
    return out


# revision 16
# speedup vs baseline: 1.0162x; 1.0162x over previous
"""MoE FFN (8 experts, top-2) on 8 Trainium2 NeuronCores.

Strategy: expert parallelism with host-side token routing.
  - Host computes the (tiny) gate: logits = x @ gate_w.T, top-2, softmax.
  - Tokens are gathered per expert and padded to a common capacity C.
  - Core e runs a dense FFN (gelu(x@W1[e].T+b1[e])@W2[e].T+b2[e]) over the
    C tokens routed to expert e, all in one SPMD Bass program.
  - Host scatters y back with the combine weights and sums the two
    expert contributions per token.

Device kernel layout (per core):
  FFN1: psum[inter128, tok] += W1T[k*128:, m*128:].T @ xT[k*128:, tok]
        h = gelu(psum + b1)           (ACT, writes bf16)
  FFN2: psum[hid128, tok]  += W2T[k*128:, m*128:].T @ h[k*128:, tok]
        y = psum + b2                 (DVE, writes f32)

DMA plumbing (v2): everything rides the two HWDGE rings (sync + scalar)
as a handful of large multi-engine DMAs.  Each dma_start is split across
all 16 SDMA engines (~340 GB/s), and each ring is FIFO in issue order,
which gives strict delivery priority: W1 column-phases then W2 k-phases
on sync; x tiles / biases / y outputs on scalar.  SWDGE (gpsimd) is not
used at all -- its Q7 descriptor rings live in SBUF and measurably slow
concurrent matmuls.  A burst of dummy matmuls at t=0 warms the PE HAM
clock (1.2 -> 2.4 GHz takes ~3.4 us of busy-ness) while the first loads
are in flight.
"""

import sys
import types

import numpy as np
import ml_dtypes

import concourse.bass as bass
import concourse.tile as tile
from concourse import mybir
from concourse.bass_utils import run_bass_kernel_spmd
from bass_rust import ScopedClock, VectorClock


def _ensure_axon_hooks():
    """run_bass_kernel_spmd(trace=True) under axon imports antenv.axon_hooks,
    which this image's antenv lacks.  Register an equivalent module backed by
    trn_agent_boot's ctypes NTFF hook so tracing works (and trace=False paths
    are unaffected)."""
    try:
        import antenv.axon_hooks  # noqa: F401
        return
    except ImportError:
        pass
    hook = None
    try:
        from trn_agent_boot.trn_boot import _ntff_profile_via_ctypes
        hook = _ntff_profile_via_ctypes("/opt/axon/libaxon_pjrt.so")
    except Exception:
        hook = None
    mod = types.ModuleType("antenv.axon_hooks")
    _state = {"hook": hook}
    mod.get_axon_ntff_profile_hook = lambda: _state["hook"]
    mod.set_axon_ntff_profile_hook = lambda h: _state.__setitem__("hook", h)
    sys.modules["antenv.axon_hooks"] = mod
    try:
        import antenv
        antenv.axon_hooks = mod
    except ImportError:
        pass


_ensure_axon_hooks()

H = 1024          # hidden
I = 4096          # intermediate
E = 8             # experts
NCORES = 8
KH = H // 128     # 8  k-tiles over hidden
KI = I // 128     # 32 k-tiles over inter
BF16 = mybir.dt.bfloat16
F32 = mybir.dt.float32

# W1 column-phases (over the 4096 inter cols).  Early phases are small so
# the first FFN1 psum-groups unblock quickly; each phase is ONE dma_start.
W1_PHASES = [(0, 128), (128, 384), (384, 640), (640, 1152), (1152, 2048),
             (2048, 3072), (3072, 4096)]
# W2 k-phases (over the 32 k-tiles of inter), consumed k-ascending.
W2_PHASES = [(0, 8), (8, 16), (16, 24), (24, 32)]

N_WARMUP_MM = 14  # 496-wide: ~3us of PE busy bridging preamble+first loads


class _TC(tile.TileContext):
    """TileContext whose tail drain splits its sem waits across SP nops.

    The walrus pinned in this container rejects a Drain instruction carrying
    more than a couple of sync waits ("Too many sync wait commands",
    CoreV3GenImpl.cpp:104).  Emit one wait-carrier nop per logical processor
    instead, then a waitless drain.
    """

    def _drain_and_barrier(self, tick_clock, wait_clock):
        nc = self.nc
        gc = tick_clock.global_clock
        ticks = eval(repr(gc).replace("VectorClock(", "").rstrip(")"))
        for i, t in enumerate(ticks):
            if t > 0:
                partial = [0] * len(ticks)
                partial[i] = t
                carrier = nc.sync.nop(nofuse=True, hint=f"drain_wait_{i}")
                wait_clock.add_sem_waits(
                    carrier.ins, ScopedClock({None: VectorClock(partial)})
                )
        nc.sync.drain()
        nc.all_engine_barrier()
        assert self.sems is not None
        popped = nc._tile_sem_poison_stack.pop()
        assert popped is self._sem_poison
        nc.clear_and_free_semaphores(list(self.sems.allocated().values()))
        nc.all_engine_barrier()


def _split_waits(nc, maxw=1):
    """The pinned walrus rejects instructions carrying more than one
    embedded sync wait ("Too many sync wait commands").  Hoist excess waits
    onto freshly inserted same-engine nops placed directly before the
    instruction -- the engine sequencer executes them in order, so the
    semantics are identical."""
    for fn in nc.m.functions:
        for bb in fn.blocks:
            new = []
            changed = False
            for inst in bb.instructions:
                si = inst.sync_info
                waits = list(si.on_wait) if si is not None else []
                if len(waits) > maxw:
                    changed = True
                    n_extra = len(waits) - maxw
                    for i in range(0, n_extra, maxw):
                        nop = mybir.InstNoOp(
                            name=nc.get_next_instruction_name(),
                            engine=inst.engine,
                            sync_info=mybir.SyncInfo(
                                on_wait=waits[i:i + maxw], on_update=[]
                            ),
                            bass_nofuse=True,
                        )
                        nc.register_instruction(nop, overwrite=True)
                        new.append(nop)
                    si.on_wait = waits[n_extra:]
                new.append(inst)
            if changed:
                bb.instructions = new


def _token_tiles(C):
    # Remainder tile last: the first (full) tile's FFN1 masks the W2 load.
    # 496-wide (not 512): a 512-col psum tile fills its bank exactly, which
    # measurably adds ~5-10 ns to every matmul in that group.
    tiles = [496] * (C // 496)
    if C % 496:
        tiles.append(C % 496)
    return tiles


def _w1_col_off(m):
    """SBUF col offset of W1 stationary block m (128 cols, one k) inside the
    phase-major w1all layout: phases concatenated, each phase laid out
    (k, cols-within-phase)."""
    off = 0
    for lo, hi in W1_PHASES:
        if m * 128 < hi:
            return off, hi - lo, m * 128 - lo
        off += KH * (hi - lo)
    raise AssertionError


def _build(C):
    """Dense per-expert FFN over C tokens; one SPMD program for all cores."""
    nc = bass.Bass()
    xt = nc.declare_dram_parameter("xt", [H, C], BF16, isOutput=False)
    w1t = nc.declare_dram_parameter("w1t", [H, I], BF16, isOutput=False)
    w2t = nc.declare_dram_parameter("w2t", [I, H], BF16, isOutput=False)
    b1 = nc.declare_dram_parameter("b1", [128, KI], F32, isOutput=False)
    b2 = nc.declare_dram_parameter("b2", [128, KH], F32, isOutput=False)
    yt = nc.declare_dram_parameter("yt", [H, C], F32, isOutput=True)

    # 3D views for single-DMA phase loads: (p, k, c) with k the 128-row block.
    w1v = w1t.rearrange("(k p) c -> p k c", k=KH)     # [128, 8, 4096]
    w2v = w2t.rearrange("(k p) c -> p k c", k=KI)     # [128, 32, 1024]
    xv = xt.rearrange("(k p) t -> p k t", k=KH)       # [128, 8, C]

    with _TC(nc) as tc:
        with (
            tc.tile_pool(name="weights", bufs=1) as wpool,
            tc.tile_pool(name="bias", bufs=1) as bpool,
            tc.tile_pool(name="x", bufs=3) as xpool,
            tc.tile_pool(name="h", bufs=1) as hpool,
            tc.tile_pool(name="o", bufs=4) as opool,
            tc.tile_pool(name="ps1", bufs=4, space="PSUM") as ps1pool,
            tc.tile_pool(name="ps2", bufs=4, space="PSUM") as ps2pool,
        ):
            # --- PE warmup: dummy matmuls on a zeroed tile so the HAM clock
            # ramps (1.2 -> 2.4 GHz) while the first real loads land.
            warm = wpool.tile([128, 624], BF16, tag="warm")
            nc.vector.memset(warm[:], 0.0)
            psw = ps1pool.tile([128, 496], F32, tag="ps1")
            for _ in range(N_WARMUP_MM):
                nc.tensor.matmul(psw[:], warm[:, 496:624], warm[:, 0:496],
                                 start=True, stop=True)

            # --- scalar(ACT) HWDGE ring: x tiles + biases (FIFO order).
            tiles = _token_tiles(C)
            xs = []

            def load_x(ti, nsplit=1):
                tw = tiles[ti]
                off = sum(tiles[:ti])
                t = xpool.tile([128, KH * tw], BF16, tag="xt")
                tv = t[:].rearrange("p (k t) -> p k t", k=KH)
                step = KH // nsplit
                for s in range(nsplit):
                    nc.scalar.dma_start(
                        tv[:, s * step:(s + 1) * step, :],
                        xv[:, s * step:(s + 1) * step, off:off + tw],
                    )
                xs.append(t)

            b1s = bpool.tile([128, KI], F32, tag="b1")
            b2s = bpool.tile([128, KH], F32, tag="b2")
            # First x tile split in four so FFN1 starts on quarter delivery;
            # b1 interleaved (needed at the first GELU, ~2us after MM 0).
            # x1/x2/b2 launches are deferred into tile-0's FFN1 so their
            # transfers don't steal HBM bandwidth from the W1 phase stream.
            tw0 = tiles[0]
            x0 = xpool.tile([128, KH * tw0], BF16, tag="xt")
            x0v = x0[:].rearrange("p (k t) -> p k t", k=KH)
            nc.scalar.dma_start(x0v[:, 0:2, :], xv[:, 0:2, 0:tw0])
            nc.scalar.dma_start(x0v[:, 2:4, :], xv[:, 2:4, 0:tw0])
            nc.scalar.dma_start(b1s[:], b1[:])
            nc.scalar.dma_start(x0v[:, 4:6, :], xv[:, 4:6, 0:tw0])
            nc.scalar.dma_start(x0v[:, 6:8, :], xv[:, 6:8, 0:tw0])
            xs.append(x0)

            # --- sync(SP) HWDGE ring: W1 column-phases then W2 k-phases.
            # Phase-major SBUF layout keeps every phase write contiguous
            # (exact dependency ranges) and every stationary block contiguous
            # (FWL-friendly).  The first two phases are split k-wise so the
            # first FFN1 psum-groups unblock on partial delivery.
            w1all = wpool.tile([128, KH * I], BF16, tag="w1")
            for pi, (lo, hi) in enumerate(W1_PHASES):
                off = sum(KH * (h_ - l_) for l_, h_ in W1_PHASES
                          if (l_, h_) < (lo, hi))
                pw = hi - lo
                nk = 2 if pi < 2 else 1
                kstep = KH // nk
                for s in range(nk):
                    dst = w1all[:, off + s * kstep * pw:
                                off + (s + 1) * kstep * pw].rearrange(
                        "p (k c) -> p k c", k=kstep)
                    nc.sync.dma_start(
                        dst, w1v[:, s * kstep:(s + 1) * kstep, lo:hi])
            w2all = wpool.tile([128, KI * H], BF16, tag="w2")
            for klo, khi in W2_PHASES:
                dst = w2all[:, klo * H:khi * H].rearrange(
                    "p (k c) -> p k c", k=khi - klo)
                nc.sync.dma_start(dst, w2v[:, klo:khi, :])

            def w1_stat(k, m):
                off, pw, rel = _w1_col_off(m)
                base = off + k * pw + rel
                return w1all[:, base:base + 128]

            off = 0
            for ti, tw in enumerate(tiles):
                xst = xs[ti]
                ht = hpool.tile([128, KI * tw], BF16, tag="h")
                for m in range(KI):
                    ps = ps1pool.tile([128, tw], F32, tag="ps1")
                    for k in range(KH):
                        nc.tensor.matmul(
                            ps[:],
                            w1_stat(k, m),
                            xst[:, k * tw:(k + 1) * tw],
                            start=(k == 0),
                            stop=(k == KH - 1),
                        )
                    nc.scalar.activation(
                        ht[:, m * tw:(m + 1) * tw],
                        ps[:],
                        mybir.ActivationFunctionType.Gelu,
                        bias=b1s[:, m:m + 1],
                    )
                    if ti == 0 and m == 8:
                        if len(tiles) > 1:
                            load_x(1)
                        nc.scalar.dma_start(b2s[:], b2[:])
                    if ti == 0 and m == 16 and len(tiles) > 2:
                        load_x(2)
                # Prefetch x for tile ti+3 AFTER this tile's FFN1: its
                # buffer WAR (xs[ti]'s last FFN1 read) is resolved by now,
                # so it doesn't block the scalar queue (GELUs/yt behind it).
                if ti + 3 <= len(tiles) - 1:
                    load_x(ti + 3)
                for m in range(KH):
                    last = ti == len(tiles) - 1 and m == KH - 1
                    # Final psum group split in column halves: half-A's
                    # ADD + DMA + HBM write receipt (~3us) hides under
                    # half-B's matmuls instead of serializing at the end.
                    halves = ([(0, tw - 160), (tw - 160, tw)]
                              if last else [(0, tw)])
                    for hj, (lo, hi) in enumerate(halves):
                        wd = hi - lo
                        ps = ps2pool.tile([128, wd], F32, tag="ps2")
                        for k in range(KI):
                            nc.tensor.matmul(
                                ps[:],
                                w2all[:, k * H + m * 128:
                                      k * H + (m + 1) * 128],
                                ht[:, k * tw + lo:k * tw + hi],
                                start=(k == 0),
                                stop=(k == KI - 1),
                            )
                        ot = opool.tile([128, wd], F32, tag="o")
                        nc.vector.tensor_scalar_add(ot[:], ps[:],
                                                    b2s[:, m:m + 1])
                        eng = nc.sync if (last and hj == 0) else nc.scalar
                        eng.dma_start(
                            yt[m * 128:(m + 1) * 128, off + lo:off + hi],
                            ot[:])
                off += tw
    _split_waits(nc)
    return nc


def _split_tiles(L):
    """Split a segment of L tokens into matmul tile widths.

    First tile 512 (masks the initial weight-phase streaming: FFN1 consumes
    W1 m-blocks slowest on a wide tile), last tile as big as possible (its
    FFN2 is the window that hides the next segment's W1 reload), middles
    >=128 (tiles narrower than ~128 risk pacing on LDWEIGHTS)."""
    if L <= 496:
        return [L]
    parts = [496]
    rem = L - 496
    while rem > 496:
        w = min(496, rem - 128)
        parts.append(w)
        rem -= w
    parts.append(rem)
    # first stays 512; order the rest ascending so the last is biggest
    return [parts[0]] + sorted(parts[1:])


def _plan_two_seg(cnts):
    """Two-segment expert-parallel plan: every core processes LA tokens of
    one expert then LB of another (weights reloaded mid-program), with
    (LA, LB) shared across cores (SPMD).  The busiest expert spans two
    A-slots, the lightest two B-slots, everyone else gets one A + one B:
      2*LA >= c_max,  LA+LB >= c_2nd,  2*LB >= c_min.
    Returns (LA, LB, slots) where slots[c] = ((eA, startA, lenA),
    (eB, startB, lenB)), or None when not profitable."""
    order = sorted(range(E), key=lambda e: -cnts[e])
    c = [cnts[e] for e in order]
    LA = -(-c[0] // 2)
    LB = max(-(-c[-1] // 2), c[1] - LA)
    LA = -(-LA // 8) * 8
    LB = max(128, -(-LB // 8) * 8)
    C1 = max(128, -(-c[0] // 128) * 128)          # single-segment capacity
    if LA + LB >= C1 or LA < 128:
        return None
    emax, emin = order[0], order[-1]
    mids = order[1:-1]                            # 6 middle experts
    a_slots = [(emax, 0), (emax, LA)] + [(e, 0) for e in mids]
    b_slots = [(e, LA) for e in mids] + [(emin, 0), (emin, LB)]
    slots = []
    for ci in range(NCORES):
        eA, sA = a_slots[ci]
        eB, sB = b_slots[ci]
        lA = max(0, min(LA, cnts[eA] - sA))
        lB = max(0, min(LB, cnts[eB] - sB))
        slots.append(((eA, sA, lA), (eB, sB, lB)))
    return LA, LB, slots


def _build_two_seg(LA, LB):
    """Per-core: segment A (LA tokens, expert A weights) then segment B
    (LB tokens, expert B weights).  B's weights stream into the same SBUF
    tiles during A's tail (WAR deps resolve per phase as A's last FFN1/FFN2
    march through the col/k ranges)."""
    tilesA = _split_tiles(LA)
    tilesB = _split_tiles(LB)
    tiles = tilesA + tilesB
    nseg_a = len(tilesA)
    C = LA + LB

    nc = bass.Bass()
    xt = nc.declare_dram_parameter("xt", [H, C], BF16, isOutput=False)
    w1ta = nc.declare_dram_parameter("w1ta", [H, I], BF16, isOutput=False)
    w2ta = nc.declare_dram_parameter("w2ta", [I, H], BF16, isOutput=False)
    b1a = nc.declare_dram_parameter("b1a", [128, KI], F32, isOutput=False)
    b2a = nc.declare_dram_parameter("b2a", [128, KH], F32, isOutput=False)
    w1tb = nc.declare_dram_parameter("w1tb", [H, I], BF16, isOutput=False)
    w2tb = nc.declare_dram_parameter("w2tb", [I, H], BF16, isOutput=False)
    b1b = nc.declare_dram_parameter("b1b", [128, KI], F32, isOutput=False)
    b2b = nc.declare_dram_parameter("b2b", [128, KH], F32, isOutput=False)
    yt = nc.declare_dram_parameter("yt", [H, C], F32, isOutput=True)

    w1va = w1ta.rearrange("(k p) c -> p k c", k=KH)
    w2va = w2ta.rearrange("(k p) c -> p k c", k=KI)
    w1vb = w1tb.rearrange("(k p) c -> p k c", k=KH)
    w2vb = w2tb.rearrange("(k p) c -> p k c", k=KI)
    xv = xt.rearrange("(k p) t -> p k t", k=KH)

    with _TC(nc) as tc:
        with (
            tc.tile_pool(name="weights", bufs=1) as wpool,
            tc.tile_pool(name="bias", bufs=1) as bpool,
            tc.tile_pool(name="x", bufs=3) as xpool,
            tc.tile_pool(name="h", bufs=1) as hpool,
            tc.tile_pool(name="o", bufs=4) as opool,
            tc.tile_pool(name="ps1", bufs=4, space="PSUM") as ps1pool,
            tc.tile_pool(name="ps2", bufs=4, space="PSUM") as ps2pool,
        ):
            warm = wpool.tile([128, 624], BF16, tag="warm")
            nc.vector.memset(warm[:], 0.0)
            psw = ps1pool.tile([128, 496], F32, tag="ps1")
            for _ in range(N_WARMUP_MM):
                nc.tensor.matmul(psw[:], warm[:, 496:624], warm[:, 0:496],
                                 start=True, stop=True)

            xs = []

            def load_x(ti, nsplit=1):
                tw = tiles[ti]
                off = sum(tiles[:ti])
                t = xpool.tile([128, KH * tw], BF16, tag="xt")
                tv = t[:].rearrange("p (k t) -> p k t", k=KH)
                step = KH // nsplit
                for s in range(nsplit):
                    nc.scalar.dma_start(
                        tv[:, s * step:(s + 1) * step, :],
                        xv[:, s * step:(s + 1) * step, off:off + tw],
                    )
                xs.append(t)

            b1sa = bpool.tile([128, KI], F32, tag="b1a")
            b2sa = bpool.tile([128, KH], F32, tag="b2a")
            b1sb = bpool.tile([128, KI], F32, tag="b1b")
            b2sb = bpool.tile([128, KH], F32, tag="b2b")
            tw0 = tiles[0]
            x0 = xpool.tile([128, KH * tw0], BF16, tag="xt")
            x0v = x0[:].rearrange("p (k t) -> p k t", k=KH)
            nc.scalar.dma_start(x0v[:, 0:2, :], xv[:, 0:2, 0:tw0])
            nc.scalar.dma_start(x0v[:, 2:4, :], xv[:, 2:4, 0:tw0])
            nc.scalar.dma_start(b1sa[:], b1a[:])
            nc.scalar.dma_start(x0v[:, 4:6, :], xv[:, 4:6, 0:tw0])
            nc.scalar.dma_start(x0v[:, 6:8, :], xv[:, 6:8, 0:tw0])
            xs.append(x0)

            w1all = wpool.tile([128, KH * I], BF16, tag="w1")
            w2all = wpool.tile([128, KI * H], BF16, tag="w2")

            def load_w(w1v, w2v, split_first=False):
                for pi, (lo, hi) in enumerate(W1_PHASES):
                    off = sum(KH * (h_ - l_) for l_, h_ in W1_PHASES
                              if (l_, h_) < (lo, hi))
                    pw = hi - lo
                    nk = 2 if (split_first and pi < 2) else 1
                    kstep = KH // nk
                    for s in range(nk):
                        dst = w1all[:, off + s * kstep * pw:
                                    off + (s + 1) * kstep * pw].rearrange(
                            "p (k c) -> p k c", k=kstep)
                        nc.sync.dma_start(
                            dst, w1v[:, s * kstep:(s + 1) * kstep, lo:hi])
                for klo, khi in W2_PHASES:
                    dst = w2all[:, klo * H:khi * H].rearrange(
                        "p (k c) -> p k c", k=khi - klo)
                    nc.sync.dma_start(dst, w2v[:, klo:khi, :])

            load_w(w1va, w2va, split_first=True)

            def w1_stat(k, m):
                off, pw, rel = _w1_col_off(m)
                base = off + k * pw + rel
                return w1all[:, base:base + 128]

            off = 0
            for ti, tw in enumerate(tiles):
                if ti == nseg_a:
                    # Segment B weights: WAR on segment A's last FFN1/FFN2
                    # reads resolves phase by phase; transfers hide under
                    # A's tail compute.
                    load_w(w1vb, w2vb)
                b1s, b2s = (b1sa, b2sa) if ti < nseg_a else (b1sb, b2sb)
                xst = xs[ti]
                ht = hpool.tile([128, KI * tw], BF16, tag="h")
                for m in range(KI):
                    ps = ps1pool.tile([128, tw], F32, tag="ps1")
                    for k in range(KH):
                        nc.tensor.matmul(
                            ps[:],
                            w1_stat(k, m),
                            xst[:, k * tw:(k + 1) * tw],
                            start=(k == 0),
                            stop=(k == KH - 1),
                        )
                    nc.scalar.activation(
                        ht[:, m * tw:(m + 1) * tw],
                        ps[:],
                        mybir.ActivationFunctionType.Gelu,
                        bias=b1s[:, m:m + 1],
                    )
                    if ti == 0 and m == 8:
                        if len(tiles) > 1:
                            load_x(1)
                        nc.scalar.dma_start(b2sa[:], b2a[:])
                    if ti == 0 and m == 16:
                        if len(tiles) > 2:
                            load_x(2)
                        nc.scalar.dma_start(b1sb[:], b1b[:])
                        nc.scalar.dma_start(b2sb[:], b2b[:])
                if ti + 3 <= len(tiles) - 1:
                    load_x(ti + 3)
                for m in range(KH):
                    last = ti == len(tiles) - 1 and m == KH - 1
                    # Final psum group split in column halves: half-A's
                    # ADD + DMA + HBM write receipt (~3us) hides under
                    # half-B's matmuls instead of serializing at the end.
                    halves = ([(0, tw - 160), (tw - 160, tw)]
                              if last else [(0, tw)])
                    for hj, (lo, hi) in enumerate(halves):
                        wd = hi - lo
                        ps = ps2pool.tile([128, wd], F32, tag="ps2")
                        for k in range(KI):
                            nc.tensor.matmul(
                                ps[:],
                                w2all[:, k * H + m * 128:
                                      k * H + (m + 1) * 128],
                                ht[:, k * tw + lo:k * tw + hi],
                                start=(k == 0),
                                stop=(k == KI - 1),
                            )
                        ot = opool.tile([128, wd], F32, tag="o")
                        nc.vector.tensor_scalar_add(ot[:], ps[:],
                                                    b2s[:, m:m + 1])
                        eng = nc.sync if (last and hj == 0) else nc.scalar
                        eng.dma_start(
                            yt[m * 128:(m + 1) * 128, off + lo:off + hi],
                            ot[:])
                off += tw
    _split_waits(nc)
    return nc


def _route(x, gate_w):
    """Host gate: top-2 of 8 logits + softmax over the selected pair."""
    logits = x @ gate_w.T                         # [T, E] f32
    T = logits.shape[0]
    rows = np.arange(T)
    i1 = np.argmax(logits, axis=1)
    v1 = logits[rows, i1]
    masked = logits.copy()
    masked[rows, i1] = -np.inf
    i2 = np.argmax(masked, axis=1)
    v2 = masked[rows, i2]
    # softmax over (v1, v2) with v1 >= v2
    e2 = np.exp(v2 - v1)
    w1 = 1.0 / (1.0 + e2)
    w2 = 1.0 - w1
    return i1, i2, w1.astype(np.float32), w2.astype(np.float32)


def _run(inputs, trace=False):
    hidden_states = np.asarray(inputs["hidden_states"], dtype=np.float32)
    gate_w = np.asarray(inputs["gate_w"], dtype=np.float32)
    W1 = np.asarray(inputs["W1"], dtype=np.float32)
    b1 = np.asarray(inputs["b1"], dtype=np.float32)
    W2 = np.asarray(inputs["W2"], dtype=np.float32)
    b2 = np.asarray(inputs["b2"], dtype=np.float32)

    B, S, _ = hidden_states.shape
    T = B * S
    x = np.ascontiguousarray(hidden_states.reshape(T, H))

    i1, i2, w1, w2 = _route(x, gate_w)
    toks = [np.flatnonzero((i1 == e) | (i2 == e)) for e in range(E)]
    cnts = [len(t) for t in toks]

    xb = x.astype(ml_dtypes.bfloat16)
    w1b = [np.ascontiguousarray(W1[e].astype(ml_dtypes.bfloat16).T)
           for e in range(E)]
    w2b = [np.ascontiguousarray(W2[e].astype(ml_dtypes.bfloat16).T)
           for e in range(E)]
    b1r = [np.ascontiguousarray(b1[e].reshape(I // 128, 128).T)
           for e in range(E)]
    b2r = [np.ascontiguousarray(b2[e].reshape(H // 128, 128).T)
           for e in range(E)]

    out = np.zeros((T, H), dtype=np.float32)

    def combine_w(e, te):
        return np.where(i1[te] == e, w1[te], w2[te])

    plan = _plan_two_seg(cnts)
    if plan is not None:
        LA, LB, slots = plan
        nc = _build_two_seg(LA, LB)
        in_maps = []
        for (eA, sA, lA), (eB, sB, lB) in slots:
            xe = np.zeros((LA + LB, H), dtype=ml_dtypes.bfloat16)
            xe[:lA] = xb[toks[eA][sA:sA + lA]]
            xe[LA:LA + lB] = xb[toks[eB][sB:sB + lB]]
            in_maps.append(
                {
                    "xt": np.ascontiguousarray(xe.T),
                    "w1ta": w1b[eA], "w2ta": w2b[eA],
                    "b1a": b1r[eA], "b2a": b2r[eA],
                    "w1tb": w1b[eB], "w2tb": w2b[eB],
                    "b1b": b1r[eB], "b2b": b2r[eB],
                }
            )
        res = run_bass_kernel_spmd(
            nc, in_maps, core_ids=list(range(NCORES)), trace=trace
        )
        for c, ((eA, sA, lA), (eB, sB, lB)) in enumerate(slots):
            ytc = res.results[c]["yt"]
            if lA:
                te = toks[eA][sA:sA + lA]
                out[te] += combine_w(eA, te)[:, None] * ytc[:, :lA].T
            if lB:
                te = toks[eB][sB:sB + lB]
                out[te] += combine_w(eB, te)[:, None] * ytc[:, LA:LA + lB].T
        return out.reshape(B, S, H), res

    C = max(128, -(-max(cnts) // 128) * 128)
    nc = _build(C)
    in_maps = []
    for e in range(E):
        xe = np.zeros((C, H), dtype=ml_dtypes.bfloat16)
        xe[: cnts[e]] = xb[toks[e]]
        in_maps.append(
            {
                "xt": np.ascontiguousarray(xe.T),
                "w1t": w1b[e], "w2t": w2b[e],
                "b1": b1r[e], "b2": b2r[e],
            }
        )
    res = run_bass_kernel_spmd(
        nc, in_maps, core_ids=list(range(NCORES)), trace=trace
    )
    for e in range(E):
        te = toks[e]
        ye = res.results[e]["yt"][:, : cnts[e]].T          # [cnt, H]
        out[te] += combine_w(e, te)[:, None] * ye
    return out.reshape(B, S, H), res


def kernel(**inputs):
    out, _ = _run(inputs, trace=False)
    return out
